# revision 4
# baseline (speedup 1.0000x reference)
"""AdaAttN sparse-attention kernel, distributed across 8 TRN2 NeuronCores.

Problem (hardcoded): B=4, C=512, H=W=64, N=4096.
  Q = f_w @ norm3(F_c_prev) + f_b ; K = g_w @ norm3(F_s_prev) + g_b
  V = h_w @ F_s + h_b
  A = softmax(Q^T K, axis=k);  M = V A^T;  S = sqrt(V^2 A^T - M^2)
  out = S * norm3(F_c) + M

Sharding: core i handles batch i//2, query half i%2 (2048 queries), with
K/V inputs replicated per batch pair.  No collectives.

Per-core pipeline (all matmuls in float32r = full-rate ~fp32-precision PE mode):
  phase 0: norm3 stats from replicated 3-channel slabs (global over batch).
  phase A: conv Q (512x2048) and K (512x4096); per 128-query block compute
           logits (q,k) bank-wise in PSUM, flash-softmax (bank max + exp with
           accumulated sums, then per-bank rescale alpha_b/s), and DMA the
           normalized A (q,k) f32 to a DRAM scratch.
  phase B: conv V^T (k,c) + V^2^T; per block: load A, PE-transpose to A^T
           (k,q), M^T = A^T^T V^T etc. accumulated over k in PSUM; epilogue
           S2 = VVA - M^2 (clamped), PE-transpose M^T/S2^T to (c,q),
           S = exp(0.5*ln(S2)) (one ACT table set for the whole kernel),
           out = S*norm3(F_c) + M.
"""

import numpy as np

import concourse.bass as bass
import concourse.mybir as mybir
from concourse import bacc
from concourse.bass import ts
from concourse.tile import TileContext
from concourse.masks import make_identity
from concourse.bass_utils import run_bass_kernel_spmd

B, C, HW = 4, 512, 4096
NQ = 2048            # queries per core
NB = NQ // 128       # 16 query blocks
CK = C // 128        # 4 channel chunks
KB = HW // 512       # 8 key banks (512 each)
KT = HW // 128       # 32 key tiles (128 each)
NSTAT = float(B * HW)  # 16384 elements per channel for norm3 stats

F32 = mybir.dt.float32
F32R = mybir.dt.float32r
AX = mybir.AxisListType.X
OP = mybir.AluOpType
AF = mybir.ActivationFunctionType

LAST_RESULTS = None
_BUILD_CACHE = {}


def _stats_block(nc, pool, psum_pool, slab_dram, sel_t, scr, out_t):
    """Compute per-channel (3) norm stats from a (12, HW) slab.

    out_t: (3, 2) sbuf tile -> col0 = 1/std (scale), col1 = -mean/std (bias).
    """
    slab = pool.tile([12, HW], F32, tag="slab")
    nc.sync.dma_start(slab[:], slab_dram[:])
    pair = pool.tile([128, 2], F32, tag="pair")
    nc.vector.memset(pair[:], 0.0)
    nc.vector.tensor_reduce(pair[0:12, 0:1], slab[:], axis=AX, op=OP.add)
    nc.scalar.activation(scr[0:12, :], slab[:], AF.Square, accum_out=pair[0:12, 1:2])
    ps = psum_pool.tile([3, 2], F32, tag="stps")
    nc.tensor.matmul(ps[:], sel_t[:], pair[:], start=True, stop=True)
    n = NSTAT
    mean = pool.tile([3, 1], F32, tag="mean")
    nc.vector.tensor_scalar_mul(mean[:], ps[:, 0:1], 1.0 / n)
    msq = pool.tile([3, 1], F32, tag="msq")
    nc.vector.tensor_tensor(msq[:], mean[:], mean[:], OP.mult)
    va = pool.tile([3, 1], F32, tag="va")
    nc.vector.tensor_scalar_mul(va[:], ps[:, 1:2], 1.0 / (n - 1.0))
    vb = pool.tile([3, 1], F32, tag="vb")
    nc.vector.tensor_scalar_mul(vb[:], msq[:], n / (n - 1.0))
    var = pool.tile([3, 1], F32, tag="var")
    nc.vector.tensor_tensor(var[:], va[:], vb[:], OP.subtract)
    lnv = pool.tile([3, 1], F32, tag="lnv")
    nc.scalar.activation(lnv[:], var[:], AF.Ln)
    # rstd = exp(-0.5 * ln(var)) = 1/sqrt(var)
    nc.scalar.activation(out_t[:, 0:1], lnv[:], AF.Exp, scale=-0.5)
    negm = pool.tile([3, 1], F32, tag="negm3")
    nc.vector.tensor_scalar_mul(negm[:], mean[:], -1.0)
    nc.vector.tensor_tensor(out_t[:, 1:2], negm[:], out_t[:, 0:1], OP.mult)


def _conv_qk(nc, tc, ctx_pool, w_r, bias_t, x_dram, norm_t, dst, width_chunks, stream_pool, psum_pool):
    """dst (128, CK, width_chunks*512) f32r = W @ norm3(x) + b."""
    x_re = x_dram[:].rearrange("(co p) q -> p co q", p=128)
    for qc in range(width_chunks):
        chunk = stream_pool.tile([128, CK, 512], F32, tag="cvf")
        nc.sync.dma_start(chunk[:], x_re[:, :, ts(qc, 512)])
        if norm_t is not None:
            nc.vector.tensor_scalar(
                chunk[0:3, 0, :], chunk[0:3, 0, :],
                norm_t[0:3, 0:1], norm_t[0:3, 1:2], OP.mult, OP.add,
            )
        chunk_r = stream_pool.tile([128, CK, 512], F32R, tag="cvr")
        nc.vector.tensor_copy(chunk_r[:], chunk[:])
        for co in range(CK):
            ps = psum_pool.tile([128, 512], F32, tag="cv")
            for ci in range(CK):
                nc.tensor.matmul(
                    ps[:], w_r[:, ci, ts(co, 128)], chunk_r[:, ci, :],
                    start=(ci == 0), stop=(ci == CK - 1),
                )
            nc.vector.tensor_scalar_add(dst[:, co, ts(qc, 512)], ps[:], bias_t[:, co:co + 1])


def build():
    if "nc" in _BUILD_CACHE:
        return _BUILD_CACHE["nc"]
    nc = bacc.Bacc()
    xq_d = nc.declare_dram_parameter("xq", [C, NQ], F32, isOutput=False)
    xk_d = nc.declare_dram_parameter("xk", [C, HW], F32, isOutput=False)
    xv_d = nc.declare_dram_parameter("xv", [C, HW], F32, isOutput=False)
    xc_d = nc.declare_dram_parameter("xc", [C, NQ], F32, isOutput=False)
    slabs_d = nc.declare_dram_parameter("slabs", [3, 12, HW], F32, isOutput=False)
    fw_d = nc.declare_dram_parameter("fw", [C, C], F32, isOutput=False)  # (c_in, c_out)
    gw_d = nc.declare_dram_parameter("gw", [C, C], F32, isOutput=False)
    hw_d = nc.declare_dram_parameter("hw", [C, C], F32, isOutput=False)
    fb_d = nc.declare_dram_parameter("fb", [128, CK], F32, isOutput=False)
    gb_d = nc.declare_dram_parameter("gb", [128, CK], F32, isOutput=False)
    hbb_d = nc.declare_dram_parameter("hbb", [128, 512], F32, isOutput=False)
    sel_d = nc.declare_dram_parameter("sel", [12, 3], F32, isOutput=False)
    out_d = nc.declare_dram_parameter("out", [C, NQ], F32, isOutput=True)

    a_scratch = nc.dram_tensor("a_scratch", [NB, 128, HW], F32)

    with TileContext(nc) as tc:
        with tc.tile_pool(name="persist", bufs=1) as persist:
            ident = persist.tile([128, 128], F32)
            make_identity(nc, ident[:])
            fb_t = persist.tile([128, CK], F32)
            nc.sync.dma_start(fb_t[:], fb_d[:])
            gb_t = persist.tile([128, CK], F32)
            nc.sync.dma_start(gb_t[:], gb_d[:])
            normq = persist.tile([3, 2], F32)
            normk = persist.tile([3, 2], F32)
            normc = persist.tile([3, 2], F32)

            # ---- phase 0: stats ----
            with (
                tc.tile_pool(name="stats", bufs=1) as stp,
                tc.tile_pool(name="stats_ps", bufs=1, space="PSUM") as stps,
            ):
                sel_t = stp.tile([128, 3], F32)
                nc.vector.memset(sel_t[:], 0.0)
                nc.sync.dma_start(sel_t[0:12, :], sel_d[:])
                scr = stp.tile([12, HW], F32)
                _stats_block(nc, stp, stps, slabs_d[0], sel_t, scr, normq)
                _stats_block(nc, stp, stps, slabs_d[1], sel_t, scr, normk)
                _stats_block(nc, stp, stps, slabs_d[2], sel_t, scr, normc)

            # ---- phase A: Q/K convs + logits + softmax -> a_scratch ----
            with tc.tile_pool(name="pa_big", bufs=1) as pa:
                q_t = pa.tile([128, CK, NQ], F32R)
                k_t = pa.tile([128, CK, HW], F32R)
                with (
                    tc.tile_pool(name="pa_w", bufs=1) as paw,
                    tc.tile_pool(name="pa_stream", bufs=2) as past,
                    tc.tile_pool(name="cv_ps", bufs=2, space="PSUM") as cvps,
                ):
                    fw_f = paw.tile([128, CK, C], F32, tag="wf")
                    nc.sync.dma_start(fw_f[:], fw_d[:].rearrange("(ci p) o -> p ci o", p=128))
                    fw_r = paw.tile([128, CK, C], F32R, tag="fwr")
                    nc.vector.tensor_copy(fw_r[:], fw_f[:])
                    gw_f = paw.tile([128, CK, C], F32, tag="wf")
                    nc.sync.dma_start(gw_f[:], gw_d[:].rearrange("(ci p) o -> p ci o", p=128))
                    gw_r = paw.tile([128, CK, C], F32R, tag="gwr")
                    nc.vector.tensor_copy(gw_r[:], gw_f[:])
                    _conv_qk(nc, tc, paw, fw_r, fb_t, xq_d, normq, q_t, NQ // 512, past, cvps)
                    _conv_qk(nc, tc, paw, gw_r, gb_t, xk_d, normk, k_t, HW // 512, past, cvps)

                with (
                    tc.tile_pool(name="lg_ps", bufs=6, space="PSUM") as lgps,
                    tc.tile_pool(name="ab", bufs=2) as abp,
                    tc.tile_pool(name="sm", bufs=2) as smp,
                ):
                    for i in range(NB):
                        ab = abp.tile([128, HW], F32, tag="ab")
                        mneg8 = smp.tile([128, KB], F32, tag="mneg8")
                        s8 = smp.tile([128, KB], F32, tag="s8")
                        for b8 in range(KB):
                            ps = lgps.tile([128, 512], F32, tag="lg")
                            for ci in range(CK):
                                nc.tensor.matmul(
                                    ps[:], q_t[:, ci, ts(i, 128)], k_t[:, ci, ts(b8, 512)],
                                    start=(ci == 0), stop=(ci == CK - 1),
                                )
                            nc.vector.tensor_reduce(
                                mneg8[:, b8:b8 + 1], ps[:], axis=AX, op=OP.max, negate=True
                            )
                            nc.scalar.activation(
                                ab[:, ts(b8, 512)], ps[:], AF.Exp,
                                bias=mneg8[:, b8:b8 + 1], accum_out=s8[:, b8:b8 + 1],
                            )
                        negm = smp.tile([128, 1], F32, tag="negm")
                        nc.vector.tensor_reduce(negm[:], mneg8[:], axis=AX, op=OP.min)
                        alpha8 = smp.tile([128, KB], F32, tag="alpha8")
                        nc.scalar.activation(alpha8[:], mneg8[:], AF.Exp, bias=negm[:, 0:1], scale=-1.0)
                        t8 = smp.tile([128, KB], F32, tag="t8")
                        nc.vector.tensor_tensor(t8[:], alpha8[:], s8[:], OP.mult)
                        ssum = smp.tile([128, 1], F32, tag="ssum")
                        nc.vector.tensor_reduce(ssum[:], t8[:], axis=AX, op=OP.add)
                        rs = smp.tile([128, 1], F32, tag="rs")
                        nc.vector.reciprocal(rs[:], ssum[:])
                        d8 = smp.tile([128, KB], F32, tag="d8")
                        nc.vector.tensor_scalar_mul(d8[:], alpha8[:], rs[:, 0:1])
                        for b8 in range(KB):
                            nc.gpsimd.tensor_scalar_mul(
                                ab[:, ts(b8, 512)], ab[:, ts(b8, 512)], d8[:, b8:b8 + 1]
                            )
                        nc.sync.dma_start(a_scratch[i], ab[:])

            # ---- phase B: V convs + M/S + epilogue ----
            with tc.tile_pool(name="pb_big", bufs=1) as pb:
                vt = pb.tile([128, KT, 512], F32R)
                vvt = pb.tile([128, KT, 512], F32R)
                with (
                    tc.tile_pool(name="pb_w", bufs=1) as pbw,
                    tc.tile_pool(name="pb_stream", bufs=2) as pbst,
                    tc.tile_pool(name="vcv_ps", bufs=2, space="PSUM") as vcps,
                ):
                    hw_f = pbw.tile([128, CK, C], F32, tag="hwf")
                    nc.sync.dma_start(hw_f[:], hw_d[:].rearrange("(ci p) o -> p ci o", p=128))
                    hw_r = pbw.tile([128, CK, C], F32R, tag="hwr")
                    nc.vector.tensor_copy(hw_r[:], hw_f[:])
                    hbb_t = pbw.tile([128, 512], F32, tag="hbb")
                    nc.sync.dma_start(hbb_t[:], hbb_d[:])
                    xv_re = xv_d[:].rearrange("(ci p) k -> p ci k", p=128)
                    for kc in range(KB):
                        chunk = pbst.tile([128, CK, 512], F32, tag="vch")
                        nc.sync.dma_start(chunk[:], xv_re[:, :, ts(kc, 512)])
                        chunk_r = pbst.tile([128, CK, 512], F32R, tag="vchr")
                        nc.vector.tensor_copy(chunk_r[:], chunk[:])
                        for kt4 in range(4):
                            kt = kc * 4 + kt4
                            ps = vcps.tile([128, 512], F32, tag="vcv")
                            for ci in range(CK):
                                nc.tensor.matmul(
                                    ps[:], chunk_r[:, ci, ts(kt4, 128)], hw_r[:, ci, :],
                                    start=(ci == 0), stop=(ci == CK - 1),
                                )
                            vb = pbst.tile([128, 512], F32, tag="vbias")
                            nc.vector.tensor_tensor(vb[:], ps[:], hbb_t[:], OP.add)
                            nc.vector.tensor_copy(vt[:, kt, :], vb[:])
                            nc.vector.tensor_tensor(vvt[:, kt, :], vb[:], vb[:], OP.mult)

                with (
                    tc.tile_pool(name="ain", bufs=2) as ainp,
                    tc.tile_pool(name="atr", bufs=1) as atrp,
                    tc.tile_pool(name="mv_ps", bufs=2, space="PSUM") as mvps,
                    tc.tile_pool(name="tp_ps", bufs=2, space="PSUM") as tpps,
                    tc.tile_pool(name="ep", bufs=1) as ep,
                    tc.tile_pool(name="ep2", bufs=2) as ep2,
                ):
                    xc_re = xc_d[:].rearrange("(co p) q -> p co q", p=128)
                    out_re = out_d[:].rearrange("(co p) q -> p co q", p=128)
                    for i in range(NB):
                        ain = ainp.tile([128, HW], F32, tag="ain")
                        nc.sync.dma_start(ain[:], a_scratch[i])
                        atr = atrp.tile([128, KT, 128], F32R, tag="atr")
                        for t in range(KT):
                            pt = tpps.tile([128, 128], F32, tag="at")
                            nc.tensor.transpose(pt[:], ain[:, ts(t, 128)], ident[:])
                            nc.vector.tensor_copy(atr[:, t, :], pt[:])
                        psm = mvps.tile([128, 512], F32, tag="m")
                        pss = mvps.tile([128, 512], F32, tag="s")
                        for kc in range(KT):
                            nc.tensor.matmul(
                                psm[:], atr[:, kc, :], vt[:, kc, :],
                                start=(kc == 0), stop=(kc == KT - 1),
                            )
                            nc.tensor.matmul(
                                pss[:], atr[:, kc, :], vvt[:, kc, :],
                                start=(kc == 0), stop=(kc == KT - 1),
                            )
                        mt = ep.tile([128, 512], F32, tag="mt")
                        nc.scalar.copy(mt[:], psm[:])
                        sq = ep.tile([128, 512], F32, tag="sq")
                        nc.vector.tensor_tensor(sq[:], mt[:], mt[:], OP.mult)
                        s2 = ep.tile([128, 512], F32, tag="s2")
                        nc.vector.tensor_tensor(s2[:], pss[:], sq[:], OP.subtract)
                        nc.vector.tensor_scalar_max(s2[:], s2[:], 1e-35)
                        xcc = ep2.tile([128, CK, 128], F32, tag="xcc")
                        nc.sync.dma_start(xcc[:], xc_re[:, :, ts(i, 128)])
                        nc.vector.tensor_scalar(
                            xcc[0:3, 0, :], xcc[0:3, 0, :],
                            normc[0:3, 0:1], normc[0:3, 1:2], OP.mult, OP.add,
                        )
                        mc = ep.tile([128, CK, 128], F32, tag="mc")
                        lnb = ep.tile([128, CK, 128], F32, tag="lnb")
                        for cc in range(CK):
                            ptm = tpps.tile([128, 128], F32, tag="ept")
                            nc.tensor.transpose(ptm[:], mt[:, ts(cc, 128)], ident[:])
                            nc.scalar.copy(mc[:, cc, :], ptm[:])
                            pts = tpps.tile([128, 128], F32, tag="ept")
                            nc.tensor.transpose(pts[:], s2[:, ts(cc, 128)], ident[:])
                            nc.scalar.activation(lnb[:, cc, :], pts[:], AF.Ln)
                        sc = ep.tile([128, CK, 128], F32, tag="sc")
                        nc.scalar.activation(sc[:], lnb[:], AF.Exp, scale=0.5)
                        ot = ep2.tile([128, CK, 128], F32, tag="ot")
                        nc.vector.tensor_tensor(ot[:], sc[:], xcc[:], OP.mult)
                        nc.vector.tensor_tensor(ot[:], ot[:], mc[:], OP.add)
                        nc.sync.dma_start(out_re[:, :, ts(i, 128)], ot[:])

    nc.compile()
    _BUILD_CACHE["nc"] = nc
    return nc


def kernel(F_c, F_s, F_c_previous, F_s_previous, f_w, f_b, g_w, g_b, h_w, h_b):
    F_c = np.ascontiguousarray(np.asarray(F_c, dtype=np.float32))
    F_s = np.ascontiguousarray(np.asarray(F_s, dtype=np.float32))
    F_c_previous = np.ascontiguousarray(np.asarray(F_c_previous, dtype=np.float32))
    F_s_previous = np.ascontiguousarray(np.asarray(F_s_previous, dtype=np.float32))
    f_w = np.asarray(f_w, dtype=np.float32)
    g_w = np.asarray(g_w, dtype=np.float32)
    h_w = np.asarray(h_w, dtype=np.float32)
    f_b = np.asarray(f_b, dtype=np.float32)
    g_b = np.asarray(g_b, dtype=np.float32)
    h_b = np.asarray(h_b, dtype=np.float32)

    fcp = F_c_previous.reshape(B, C, HW)
    fsp = F_s_previous.reshape(B, C, HW)
    fs = F_s.reshape(B, C, HW)
    fc = F_c.reshape(B, C, HW)

    slabs = np.ascontiguousarray(
        np.stack([
            fcp[:, :3].reshape(B * 3, HW),
            fsp[:, :3].reshape(B * 3, HW),
            fc[:, :3].reshape(B * 3, HW),
        ])
    )
    sel = np.zeros((12, 3), np.float32)
    for p in range(12):
        sel[p, p % 3] = 1.0
    common = {
        "slabs": slabs,
        "fw": np.ascontiguousarray(f_w.T),
        "gw": np.ascontiguousarray(g_w.T),
        "hw": np.ascontiguousarray(h_w.T),
        "fb": np.ascontiguousarray(f_b.reshape(CK, 128).T),
        "gb": np.ascontiguousarray(g_b.reshape(CK, 128).T),
        "hbb": np.ascontiguousarray(np.broadcast_to(h_b, (128, 512))),
        "sel": sel,
    }
    in_maps = []
    for i in range(8):
        b = i // 2
        qs = slice((i % 2) * NQ, (i % 2) * NQ + NQ)
        in_maps.append({
            "xq": np.ascontiguousarray(fcp[b][:, qs]),
            "xk": fsp[b],
            "xv": fs[b],
            "xc": np.ascontiguousarray(fc[b][:, qs]),
            **common,
        })

    nc = build()
    res = run_bass_kernel_spmd(nc, in_maps, core_ids=list(range(8)))
    global LAST_RESULTS
    LAST_RESULTS = res

    out = np.empty((B, C, HW), np.float32)
    for i in range(8):
        b = i // 2
        qs = slice((i % 2) * NQ, (i % 2) * NQ + NQ)
        out[b][:, qs] = res.results[i]["out"]
    return out
